# revision 5
# baseline (speedup 1.0000x reference)
"""MoE top-2 routing kernel for 8 Trainium2 NeuronCores.

Problem (hardcoded shapes): x [64,8,2048] f32, gate_w [2048,8] f32,
w1/w3 [8,2048,4096] f32, w2 [8,4096,2048] f32, top_k=2.

Strategy (expert parallelism):
  - Host computes the gate (512x8 logits, top-2, softmax) exactly as the
    reference does -- ~17 MFLOP, negligible.
  - Tokens are dispatched per expert (gathered + padded to capacity C),
    one expert per NeuronCore.  Each core runs the SwiGLU FFN for its
    expert over its C token slots:
        outT = w2^T @ (silu(w1^T @ xT) * (w3^T @ xT))
    with all matmuls laid out [K, M]/[K, N] so no on-device transposes
    are needed (tokens are the moving free dim).
  - Combine weights are folded into the host-side scatter-add of the
    per-expert outputs back into the [512, 2048] output.

The kernel is HBM-bandwidth-bound on the expert weights (50.3 MB bf16
per core, ~140 us at 358 GB/s).  Design choices to stay at that floor:
  - C = roundup(max tokens per expert) (144 for the reference routing),
    so tensor-engine busy (~96 us) sits well under the DMA floor.
  - Stage-1 groups produce 256 F-columns each (G1=2: 2 gate + 2 up PSUM
    banks), so consecutive groups ping-pong between PSUM banks 0-3/4-7
    and the PE never waits for the silu/mult drain.
  - Stage-2 groups produce 512 D-rows each (G2=4), same ping-pong.
  - Weight DMAs move [128, 4KB] lines (4 k-tiles per transfer, 512 KB)
    alternating across the two HWDGE queues (sync/scalar); deep tile
    rings keep stage-2 weight loads issued ~25 us before stage 1 ends
    so the DMA queues never drain at the stage boundary.
  - outT is stored as bf16 (host upcasts and combines in f32).
"""

import numpy as np

B, S, D, F, E = 64, 8, 2048, 4096, 8
T = B * S     # 512 tokens
P = 128
KD = D // P   # 16 k-tiles, D contraction
KF = F // P   # 32 k-tiles, F contraction
G1 = 2        # stage-1 m-tiles per group (2 gate + 2 up PSUM banks)
NG1 = F // (G1 * P)   # 16 stage-1 groups
G2 = 4        # stage-2 m-tiles per group (4 PSUM banks)
NG2 = D // (G2 * P)   # 4 stage-2 groups
KK = 4        # k-tiles per weight DMA transfer (4KB partition lines)

W1_BUFS = 16  # stage-1 weight ring (4KB/partition each)
W2_BUFS = 16  # stage-2 weight ring

_cache = {}
last_results = None  # BassKernelResults of the most recent device run


def _build(C):
    import concourse.mybir as mybir
    import concourse.tile as tile
    from concourse import bacc

    nc = bacc.Bacc(None, target_bir_lowering=False)
    f32 = mybir.dt.float32
    bf16 = mybir.dt.bfloat16

    # weights packed on host so each dma_start moves one [128, 4KB] block:
    w13 = nc.declare_dram_parameter("w13", [NG1, KD // KK, P, KK, 2, G1 * P],
                                    bf16, isOutput=False)
    w2p = nc.declare_dram_parameter("w2p", [NG2, KF // KK, P, KK, G2 * P],
                                    bf16, isOutput=False)
    xT = nc.declare_dram_parameter("xT", [P, KD, C], bf16, isOutput=False)
    outT = nc.declare_dram_parameter("outT", [NG2, P, G2, C], bf16,
                                     isOutput=True)

    with tile.TileContext(nc) as tc:
        with (
            tc.tile_pool(name="xpool", bufs=1) as xpool,
            tc.tile_pool(name="hpool", bufs=1) as hpool,
            tc.tile_pool(name="wpool", bufs=W1_BUFS) as wpool,
            tc.tile_pool(name="wpool2", bufs=W2_BUFS) as wpool2,
            tc.tile_pool(name="psum", bufs=8, space="PSUM") as psum,
            tc.tile_pool(name="spool", bufs=4) as spool,
            tc.tile_pool(name="opool", bufs=2) as opool,
        ):
            dma_eng = [nc.sync, nc.scalar]
            ndma = 0

            xt = xpool.tile([P, KD, C], bf16)
            for i in range(4):
                dma_eng[i % 2].dma_start(out=xt[:, 4 * i:4 * i + 4, :],
                                         in_=xT[:, 4 * i:4 * i + 4, :])
            ht = hpool.tile([P, KF, C], bf16)

            # PE clock ramp-up (HAM gate): ~3.4us of matmul activity
            warm = xpool.tile([P, 256], bf16, name="warm")
            nc.vector.memset(warm[:], 0.0)
            ps_w = psum.tile([P, C], f32, tag="ps", name="ps_warm")
            for i in range(32):
                nc.tensor.matmul(ps_w[:], warm[:, :P], warm[:, :C],
                                 start=True, stop=True)

            # stage 1: ht[f, t] = silu(w1^T xT) * (w3^T xT), F-major groups
            for g in range(NG1):
                ps_g = [psum.tile([P, C], f32, tag="ps", name=f"ps_g{g}_{m}")
                        for m in range(G1)]
                ps_u = [psum.tile([P, C], f32, tag="ps", name=f"ps_u{g}_{m}")
                        for m in range(G1)]
                for kp in range(KD // KK):
                    wt = wpool.tile([P, KK, 2, G1 * P], bf16, tag="w")
                    dma_eng[ndma % 2].dma_start(out=wt[:], in_=w13[g, kp])
                    ndma += 1
                    for kk in range(KK):
                        k = kp * KK + kk
                        st, sp = (k == 0), (k == KD - 1)
                        for m in range(G1):
                            nc.tensor.matmul(
                                ps_g[m][:], wt[:, kk, 0, m * P:(m + 1) * P],
                                xt[:, k, :], start=st, stop=sp)
                            nc.tensor.matmul(
                                ps_u[m][:], wt[:, kk, 1, m * P:(m + 1) * P],
                                xt[:, k, :], start=st, stop=sp)
                for m in range(G1):
                    sig = spool.tile([P, C], f32, tag="sig")
                    nc.scalar.activation(sig[:], ps_g[m][:],
                                         mybir.ActivationFunctionType.Silu)
                    nc.vector.tensor_tensor(out=ht[:, g * G1 + m, :],
                                            in0=sig[:], in1=ps_u[m][:],
                                            op=mybir.AluOpType.mult)

            # stage 2: outT[d, t] = w2^T @ hT
            for g in range(NG2):
                ps_o = [psum.tile([P, C], f32, tag="ps", name=f"ps_o{g}_{m}")
                        for m in range(G2)]
                for kp in range(KF // KK):
                    wt = wpool2.tile([P, KK, G2 * P], bf16, tag="w2")
                    dma_eng[ndma % 2].dma_start(out=wt[:], in_=w2p[g, kp])
                    ndma += 1
                    for kk in range(KK):
                        k = kp * KK + kk
                        st, sp = (k == 0), (k == KF - 1)
                        for m in range(G2):
                            nc.tensor.matmul(
                                ps_o[m][:], wt[:, kk, m * P:(m + 1) * P],
                                ht[:, k, :], start=st, stop=sp)
                obuf = opool.tile([P, G2, C], bf16, tag="o", name=f"ob{g}")
                for m in range(G2):
                    nc.vector.tensor_copy(out=obuf[:, m, :], in_=ps_o[m][:])
                if g < NG2 - 1:
                    dma_eng[ndma % 2].dma_start(out=outT[g], in_=obuf[:])
                    ndma += 1
                else:
                    # final group: split the store so the first half
                    # dispatches while the last casts drain.
                    nc.sync.dma_start(out=outT[g, :, 0:2, :],
                                      in_=obuf[:, 0:2, :])
                    nc.sync.dma_start(out=outT[g, :, 2:4, :],
                                      in_=obuf[:, 2:4, :])

    nc.compile()
    return nc


def _route(x2d, gate_w, top_k):
    """Replicates the reference gate on host: returns (sel [T,k], cw [T,k])."""
    logits = x2d @ gate_w                       # [T, E] fp32
    sel = np.argsort(-logits, axis=-1, kind="stable")[:, :top_k]
    vals = np.take_along_axis(logits, sel, axis=-1)
    m = vals.max(axis=-1, keepdims=True)
    ex = np.exp(vals - m)
    cw = ex / ex.sum(axis=-1, keepdims=True)
    return sel, cw


def kernel(x, gate_w, w1, w3, w2, top_k):
    import ml_dtypes
    from concourse.bass_utils import run_bass_kernel_spmd

    bf16 = np.dtype(ml_dtypes.bfloat16)
    x = np.asarray(x, np.float32)
    gate_w = np.asarray(gate_w, np.float32)
    w1 = np.asarray(w1, np.float32)
    w3 = np.asarray(w3, np.float32)
    w2 = np.asarray(w2, np.float32)
    k = int(top_k)

    x2d = x.reshape(T, D)
    sel, cw = _route(x2d, gate_w, k)

    # token lists per expert
    idx = [np.where((sel == e).any(axis=1))[0] for e in range(E)]
    wgt = []
    for e in range(E):
        m = sel[idx[e]] == e
        wgt.append(cw[idx[e]][m].astype(np.float32))
    counts = np.array([len(i) for i in idx])
    maxc = int(counts.max())
    C = max(96, -(-maxc // 16) * 16)
    n_chunks = 1
    if C > 512:  # capacity overflow: run multiple passes of 512
        C = 512
        n_chunks = -(-maxc // C)

    if C not in _cache:
        _cache[C] = _build(C)
    nc = _cache[C]

    wpacked = []
    for e in range(E):
        # w13 [NG1, KD//KK, P, kk, w, G1*P]: line = one 4KB block/partition
        # d = (kp*KK + kk)*P + p ; f = g*(G1*P) + col
        w1r = w1[e].astype(bf16).reshape(KD // KK, KK, P, NG1, G1 * P)
        w3r = w3[e].astype(bf16).reshape(KD // KK, KK, P, NG1, G1 * P)
        w13 = np.ascontiguousarray(
            np.stack([w1r, w3r], axis=4).transpose(3, 0, 2, 1, 4, 5))
        # w2p [NG2, KF//KK, P, kk, G2*P]: f = (kp*KK+kk)*P + p ; d = g*512+col
        w2r = w2[e].astype(bf16).reshape(KF // KK, KK, P, NG2, G2 * P)
        w2pk = np.ascontiguousarray(w2r.transpose(3, 0, 2, 1, 4))
        wpacked.append((w13, w2pk))

    out = np.zeros((T, D), np.float32)
    for chunk in range(n_chunks):
        in_maps = []
        for e in range(E):
            ide = idx[e][chunk * C:(chunk + 1) * C]
            xTe = np.zeros((D, C), bf16)
            xTe[:, :len(ide)] = x2d[ide].T.astype(bf16)
            in_maps.append({
                "xT": np.ascontiguousarray(
                    xTe.reshape(KD, P, C).transpose(1, 0, 2)),
                "w13": wpacked[e][0],
                "w2p": wpacked[e][1],
            })
        res = run_bass_kernel_spmd(nc, in_maps, core_ids=list(range(E)))
        global last_results
        last_results = res
        for e in range(E):
            ide = idx[e][chunk * C:(chunk + 1) * C]
            if len(ide) == 0:
                continue
            we = wgt[e][chunk * C:(chunk + 1) * C]
            # outT [NG2, P, G2, C] -> [D, C] with d = g*G2*P + m*P + p
            oTe = res.results[e]["outT"].astype(np.float32)
            oTe = oTe.transpose(0, 2, 1, 3).reshape(D, C)
            # token indices are unique within one expert's list
            out[ide] += we[:, None] * oTe[:, :len(ide)].T

    return out.reshape(B, S, D)


# revision 11
# speedup vs baseline: 1.0028x; 1.0028x over previous
"""MoE top-2 routing kernel for 8 Trainium2 NeuronCores.

Problem (hardcoded shapes): x [64,8,2048] f32, gate_w [2048,8] f32,
w1/w3 [8,2048,4096] f32, w2 [8,4096,2048] f32, top_k=2.

Strategy (expert parallelism):
  - Host computes the gate (512x8 logits, top-2, softmax) exactly as the
    reference does -- ~17 MFLOP, negligible.
  - Tokens are dispatched per expert (gathered + padded to capacity C),
    one expert per NeuronCore.  Each core runs the SwiGLU FFN for its
    expert over its C token slots:
        outT = w2^T @ (silu(w1^T @ xT) * (w3^T @ xT))
    with all matmuls laid out [K, M]/[K, N] so no on-device transposes
    are needed (tokens are the moving free dim).
  - Combine weights are folded into the host-side scatter-add of the
    per-expert outputs back into the [512, 2048] output.

The kernel is HBM-bandwidth-bound on the expert weights (50.3 MB bf16
per core, ~140 us at 358 GB/s).  Design choices to stay at that floor:
  - C = roundup(max tokens per expert) (144 for the reference routing),
    so tensor-engine busy (~96 us) sits well under the DMA floor.
  - Stage-1 groups produce 256 F-columns each (G1=2: 2 gate + 2 up PSUM
    banks), so consecutive groups ping-pong between PSUM banks 0-3/4-7
    and the PE never waits for the silu/mult drain.
  - Stage-2 groups produce 512 D-rows each (G2=4), same ping-pong.
  - Weight DMAs move [128, 4KB] lines (4 k-tiles per transfer, 512 KB)
    alternating across the two HWDGE queues (sync/scalar); deep tile
    rings keep stage-2 weight loads issued ~25 us before stage 1 ends
    so the DMA queues never drain at the stage boundary.
  - outT is stored as bf16 (host upcasts and combines in f32).
"""

import numpy as np

B, S, D, F, E = 64, 8, 2048, 4096, 8
T = B * S     # 512 tokens
P = 128
KD = D // P   # 16 k-tiles, D contraction
KF = F // P   # 32 k-tiles, F contraction
G1 = 2        # stage-1 m-tiles per group (2 gate + 2 up PSUM banks)
NG1 = F // (G1 * P)   # 16 stage-1 groups
G2 = 4        # stage-2 m-tiles per group (4 PSUM banks)
NG2 = D // (G2 * P)   # 4 stage-2 groups
KK = 4        # k-tiles per weight DMA transfer (4KB partition lines)

W1_BUFS = 16  # stage-1 weight ring (4KB/partition each)
W2_BUFS = 16  # stage-2 weight ring

_cache = {}
last_results = None  # BassKernelResults of the most recent device run


def _build(C):
    import concourse.mybir as mybir
    import concourse.tile as tile
    from concourse import bacc

    nc = bacc.Bacc(None, target_bir_lowering=False)
    f32 = mybir.dt.float32
    bf16 = mybir.dt.bfloat16

    # weights packed on host so each dma_start moves one [128, 4KB] block:
    w13 = nc.declare_dram_parameter("w13", [NG1, KD // KK, P, KK, 2, G1 * P],
                                    bf16, isOutput=False)
    w2p = nc.declare_dram_parameter("w2p", [NG2, KF // KK, P, KK, G2 * P],
                                    bf16, isOutput=False)
    xT = nc.declare_dram_parameter("xT", [P, KD, C], bf16, isOutput=False)
    outT = nc.declare_dram_parameter("outT", [P, NG2, G2, C], bf16,
                                     isOutput=True)

    with tile.TileContext(nc) as tc:
        with (
            tc.tile_pool(name="xpool", bufs=1) as xpool,
            tc.tile_pool(name="hpool", bufs=1) as hpool,
            tc.tile_pool(name="wpool", bufs=W1_BUFS) as wpool,
            tc.tile_pool(name="wpool2", bufs=W2_BUFS) as wpool2,
            tc.tile_pool(name="psum", bufs=8, space="PSUM") as psum,
            tc.tile_pool(name="spool", bufs=4) as spool,
            tc.tile_pool(name="opool", bufs=1) as opool,
        ):
            dma_eng = [nc.sync, nc.scalar]
            ndma = 0

            xt = xpool.tile([P, KD, C], bf16)
            for i in range(4):
                dma_eng[i % 2].dma_start(out=xt[:, 4 * i:4 * i + 4, :],
                                         in_=xT[:, 4 * i:4 * i + 4, :])
            ht = hpool.tile([P, KF, C], bf16)

            # PE clock ramp-up (HAM gate): ~3.4us of matmul activity
            warm = xpool.tile([P, 256], bf16, name="warm")
            nc.vector.memset(warm[:], 0.0)
            ps_w = psum.tile([P, C], f32, tag="ps", name="ps_warm")
            for i in range(32):
                nc.tensor.matmul(ps_w[:], warm[:, :P], warm[:, :C],
                                 start=True, stop=True)

            # stage 1: ht[f, t] = silu(w1^T xT) * (w3^T xT), F-major groups
            for g in range(NG1):
                ps_g = [psum.tile([P, C], f32, tag="ps", name=f"ps_g{g}_{m}")
                        for m in range(G1)]
                ps_u = [psum.tile([P, C], f32, tag="ps", name=f"ps_u{g}_{m}")
                        for m in range(G1)]
                for kp in range(KD // KK):
                    wt = wpool.tile([P, KK, 2, G1 * P], bf16, tag="w")
                    dma_eng[ndma % 2].dma_start(out=wt[:], in_=w13[g, kp])
                    ndma += 1
                    for kk in range(KK):
                        k = kp * KK + kk
                        st, sp = (k == 0), (k == KD - 1)
                        for m in range(G1):
                            nc.tensor.matmul(
                                ps_g[m][:], wt[:, kk, 0, m * P:(m + 1) * P],
                                xt[:, k, :], start=st, stop=sp)
                            nc.tensor.matmul(
                                ps_u[m][:], wt[:, kk, 1, m * P:(m + 1) * P],
                                xt[:, k, :], start=st, stop=sp)
                for m in range(G1):
                    sig = spool.tile([P, C], f32, tag="sig")
                    nc.scalar.activation(sig[:], ps_g[m][:],
                                         mybir.ActivationFunctionType.Silu)
                    nc.vector.tensor_tensor(out=ht[:, g * G1 + m, :],
                                            in0=sig[:], in1=ps_u[m][:],
                                            op=mybir.AluOpType.mult)

            # stage 2: outT[d, t] = w2^T @ hT
            # All output accumulates in one SBUF tile; the stores are
            # emitted only after the last weight dma_start on each engine,
            # so slow store completions (HBM write receipt ~2us) never
            # land on a semaphore lane that a weight-tile wait also uses.
            obig = opool.tile([P, NG2, G2, C], bf16, name="obig")
            for g in range(NG2):
                ps_o = [psum.tile([P, C], f32, tag="ps", name=f"ps_o{g}_{m}")
                        for m in range(G2)]
                for kp in range(KF // KK):
                    wt = wpool2.tile([P, KK, G2 * P], bf16, tag="w2")
                    dma_eng[ndma % 2].dma_start(out=wt[:], in_=w2p[g, kp])
                    ndma += 1
                    for kk in range(KK):
                        k = kp * KK + kk
                        st, sp = (k == 0), (k == KF - 1)
                        for m in range(G2):
                            nc.tensor.matmul(
                                ps_o[m][:], wt[:, kk, m * P:(m + 1) * P],
                                ht[:, k, :], start=st, stop=sp)
                for m in range(G2):
                    nc.vector.tensor_copy(out=obig[:, g, m, :],
                                          in_=ps_o[m][:])
            nc.sync.dma_start(out=outT[:, 0:NG2 - 1], in_=obig[:, 0:NG2 - 1])
            nc.scalar.dma_start(out=outT[:, NG2 - 1], in_=obig[:, NG2 - 1])

    nc.compile()
    return nc


def _route(x2d, gate_w, top_k):
    """Replicates the reference gate on host: returns (sel [T,k], cw [T,k])."""
    logits = x2d @ gate_w                       # [T, E] fp32
    sel = np.argsort(-logits, axis=-1, kind="stable")[:, :top_k]
    vals = np.take_along_axis(logits, sel, axis=-1)
    m = vals.max(axis=-1, keepdims=True)
    ex = np.exp(vals - m)
    cw = ex / ex.sum(axis=-1, keepdims=True)
    return sel, cw


def kernel(x, gate_w, w1, w3, w2, top_k):
    import ml_dtypes
    from concourse.bass_utils import run_bass_kernel_spmd

    bf16 = np.dtype(ml_dtypes.bfloat16)
    x = np.asarray(x, np.float32)
    gate_w = np.asarray(gate_w, np.float32)
    w1 = np.asarray(w1, np.float32)
    w3 = np.asarray(w3, np.float32)
    w2 = np.asarray(w2, np.float32)
    k = int(top_k)

    x2d = x.reshape(T, D)
    sel, cw = _route(x2d, gate_w, k)

    # token lists per expert
    idx = [np.where((sel == e).any(axis=1))[0] for e in range(E)]
    wgt = []
    for e in range(E):
        m = sel[idx[e]] == e
        wgt.append(cw[idx[e]][m].astype(np.float32))
    counts = np.array([len(i) for i in idx])
    maxc = int(counts.max())
    C = max(96, -(-maxc // 16) * 16)
    n_chunks = 1
    if C > 512:  # capacity overflow: run multiple passes of 512
        C = 512
        n_chunks = -(-maxc // C)

    if C not in _cache:
        _cache[C] = _build(C)
    nc = _cache[C]

    wpacked = []
    for e in range(E):
        # w13 [NG1, KD//KK, P, kk, w, G1*P]: line = one 4KB block/partition
        # d = (kp*KK + kk)*P + p ; f = g*(G1*P) + col
        w1r = w1[e].astype(bf16).reshape(KD // KK, KK, P, NG1, G1 * P)
        w3r = w3[e].astype(bf16).reshape(KD // KK, KK, P, NG1, G1 * P)
        w13 = np.ascontiguousarray(
            np.stack([w1r, w3r], axis=4).transpose(3, 0, 2, 1, 4, 5))
        # w2p [NG2, KF//KK, P, kk, G2*P]: f = (kp*KK+kk)*P + p ; d = g*512+col
        w2r = w2[e].astype(bf16).reshape(KF // KK, KK, P, NG2, G2 * P)
        w2pk = np.ascontiguousarray(w2r.transpose(3, 0, 2, 1, 4))
        wpacked.append((w13, w2pk))

    out = np.zeros((T, D), np.float32)
    for chunk in range(n_chunks):
        in_maps = []
        for e in range(E):
            ide = idx[e][chunk * C:(chunk + 1) * C]
            xTe = np.zeros((D, C), bf16)
            xTe[:, :len(ide)] = x2d[ide].T.astype(bf16)
            in_maps.append({
                "xT": np.ascontiguousarray(
                    xTe.reshape(KD, P, C).transpose(1, 0, 2)),
                "w13": wpacked[e][0],
                "w2p": wpacked[e][1],
            })
        res = run_bass_kernel_spmd(nc, in_maps, core_ids=list(range(E)))
        global last_results
        last_results = res
        for e in range(E):
            ide = idx[e][chunk * C:(chunk + 1) * C]
            if len(ide) == 0:
                continue
            we = wgt[e][chunk * C:(chunk + 1) * C]
            # outT [P, NG2, G2, C] -> [D, C] with d = g*G2*P + m*P + p
            oTe = res.results[e]["outT"].astype(np.float32)
            oTe = oTe.transpose(1, 2, 0, 3).reshape(D, C)
            # token indices are unique within one expert's list
            out[ide] += we[:, None] * oTe[:, :len(ide)].T

    return out.reshape(B, S, D)
